# revision 49
# baseline (speedup 1.0000x reference)
"""Trainium2 Bass kernel for nn_DAGConcat (DAG-GNN + LSTM + MLP).

Sharding: data-parallel over B=32 dialogues across 8 cores (4 each).
The per-utterance GNN recurrence (sequential in N=128, x2 layers) runs
per-core on its 4 dialogues; the LSTM (sequential over B due to the
batch_first quirk) is replicated; big projections are B-sharded.
"""
import os
import sys

for _p in ('/opt/trn_rl_repo', '/root/.axon_site/_ro/trn_rl_repo'):
    if os.path.isdir(_p) and _p not in sys.path:
        sys.path.insert(0, _p)

import numpy as np
import concourse.bass as bass
import concourse.mybir as mybir
import concourse.tile as tile
from concourse.bass_utils import run_bass_kernel_spmd
from concourse.masks import make_identity

F32 = mybir.dt.float32
BF16 = mybir.dt.bfloat16
I32 = mybir.dt.int32
AF = mybir.ActivationFunctionType
ALU = mybir.AluOpType
AX = mybir.AxisListType

B, N, EMB, HID, L, NCLS = 32, 128, 1024, 512, 2, 7
NCORES = 8
BL = B // NCORES
H4, H2 = 4 * HID, 2 * HID
PRE = 6 * HID + 1      # pre-MM cols: sigma(2048) | nC(512) | nP(512) | Qpre(1)
DEBUG = bool(int(os.environ.get('KDEBUG', '0')))
KSKIP_LSTM = bool(int(os.environ.get('KSKIP_LSTM', '0')))
KSKIP_REC = bool(int(os.environ.get('KSKIP_REC', '0')))
KSKIP_PRE = bool(int(os.environ.get('KSKIP_PRE', '0')))
KNSTEP = int(os.environ.get('KNSTEP', str(N)))
KSKIP_MLP = bool(int(os.environ.get('KSKIP_MLP', '0')))
KSKIP_H0 = bool(int(os.environ.get('KSKIP_H0', '0')))
KSIMINIT = bool(int(os.environ.get('KSIMINIT', '0')))


def _patch_drain():
    """Kernel-tail drain waits on >limit sems -> emit 1-wait drains instead."""
    from concourse.vector_clock import ScopedClock, VectorClock

    def _drain_and_barrier(self, tick_clock, wait_clock):
        gc = tick_clock.global_clock
        n = len(gc)
        for st in range(n):
            if gc[st] <= 0:
                continue
            vec = [0] * n
            vec[st] = gc[st]
            di = self.nc.sync.drain()
            wait_clock.add_sem_waits(di.ins, ScopedClock({None: VectorClock(vec)}))
        self.nc.all_engine_barrier()
        popped = self.nc._tile_sem_poison_stack.pop()
        assert popped is self._sem_poison
        self.nc.clear_and_free_semaphores(list(self.sems.allocated().values()))
        self.nc.all_engine_barrier()

    tile.TileContext._drain_and_barrier = _drain_and_barrier


_patch_drain()


# ================================================================ host prep

def prep_inputs(inp):
    f4 = np.float32
    feats = np.asarray(inp['features'], f4)
    adj = np.asarray(inp['adj'], f4)
    s_mask = np.asarray(inp['s_mask'], f4)

    g = {}
    # LSTM: gate col order [i f o g] so sigma gates are contiguous
    perm = np.concatenate([np.arange(0, HID), np.arange(HID, 2 * HID),
                           np.arange(3 * HID, 4 * HID), np.arange(2 * HID, 3 * HID)])
    g['lstm_WihT'] = np.ascontiguousarray(np.asarray(inp['lstm_Wih'], f4).T[:, perm])
    g['lstm_WhhT'] = np.ascontiguousarray(np.asarray(inp['lstm_Whh'], f4).T[:, perm])
    g['lstm_b'] = (np.asarray(inp['lstm_bih'], f4)
                   + np.asarray(inp['lstm_bhh'], f4))[perm][None, :]
    g['fc1T'] = np.ascontiguousarray(np.asarray(inp['fc1_W'], f4).T)
    g['fc1_b'] = np.asarray(inp['fc1_b'], f4)[None, :]

    for l in range(L):
        aW = np.asarray(inp['attn_W'][l], f4)
        wq, wk = aW[:HID], aW[HID:]
        ab = float(np.asarray(inp['attn_b'], f4)[l])
        cWihT = np.asarray(inp['gruC_Wih'][l], f4).T
        cWhhT = np.asarray(inp['gruC_Whh'][l], f4).T
        cbih = np.asarray(inp['gruC_bih'][l], f4)
        cbhh = np.asarray(inp['gruC_bhh'][l], f4)
        pWihT = np.asarray(inp['gruP_Wih'][l], f4).T
        pWhhT = np.asarray(inp['gruP_Whh'][l], f4).T
        pbih = np.asarray(inp['gruP_bih'][l], f4)
        pbhh = np.asarray(inp['gruP_bhh'][l], f4)
        Wr0 = np.asarray(inp['Wr0'][l], f4)
        Wr1 = np.asarray(inp['Wr1'][l], f4)
        r, z, n_ = slice(0, HID), slice(HID, 2 * HID), slice(2 * HID, 3 * HID)
        g[f'Wc1_{l}'] = np.ascontiguousarray(np.concatenate(
            [cWhhT[:, r], cWhhT[:, z], pWihT[:, r], pWihT[:, z]], axis=1))
        g[f'Wc2_{l}'] = np.ascontiguousarray(np.concatenate(
            [cWhhT[:, n_], pWihT[:, n_]], axis=1))
        g[f'bias2_{l}'] = np.concatenate([cbhh[n_], pbih[n_]])[None, :]
        g[f'Wr_{l}'] = np.ascontiguousarray(np.concatenate(
            [Wr0.T, Wr1.T, -wk[:, None]], axis=1))
        g[f'Wpre_{l}'] = np.ascontiguousarray(np.concatenate(
            [cWihT[:, r], cWihT[:, z], pWhhT[:, r], pWhhT[:, z],
             cWihT[:, n_], pWhhT[:, n_], wq[:, None]], axis=1))
        g[f'biaspre_{l}'] = np.concatenate(
            [cbih[r] + cbhh[r], cbih[z] + cbhh[z], pbih[r] + pbhh[r],
             pbih[z] + pbhh[z], cbih[n_], pbhh[n_], [ab]])[None, :].astype(f4)

    g['mlp0T'] = np.ascontiguousarray(np.asarray(inp['mlp0_W'], f4).T)
    g['mlp0_b'] = np.asarray(inp['mlp0_b'], f4)[None, :]
    g['mlp1T'] = np.ascontiguousarray(np.asarray(inp['mlp1_W'], f4).T)
    g['mlp1_b'] = np.asarray(inp['mlp1_b'], f4)[None, :]
    ow = np.zeros((HID, 8), f4)
    ow[:, :NCLS] = np.asarray(inp['out_W'], f4).T
    g['outWT'] = ow
    ob = np.zeros((1, 8), f4)
    ob[0, :NCLS] = np.asarray(inp['out_b'], f4)
    g['out_b'] = ob

    featT = np.ascontiguousarray(feats.transpose(2, 0, 1))  # [EMB, B, N]
    g['featT_full'] = featT

    maps = []
    for c in range(NCORES):
        bs = slice(BL * c, BL * (c + 1))
        m = dict(g)
        m['featT_l'] = np.ascontiguousarray(featT[:, bs, :])
        m['adjbias'] = np.ascontiguousarray((adj[bs] - 1.0) * 1e30)
        m['sT'] = np.ascontiguousarray(s_mask[bs].transpose(2, 1, 0))
        # sdiagT[b, i] = s_mask[b, i, i-1]: the s value for the lag-1
        # attention-correction term at step i
        sd = np.zeros((BL, N), f4)
        sd[:, 1:] = s_mask[bs][:, np.arange(1, N), np.arange(0, N - 1)]
        m['sdiagT'] = sd
        m['town'] = np.arange(BL * c, BL * (c + 1), dtype=np.int32)[None, :]
        maps.append(m)
    return maps


SHAPES = {
    'lstm_WihT': (EMB, 4 * HID), 'lstm_WhhT': (HID, 4 * HID), 'lstm_b': (1, 4 * HID),
    'fc1T': (EMB, HID), 'fc1_b': (1, HID),
    'mlp0T': (4 * HID + EMB, HID), 'mlp0_b': (1, HID),
    'mlp1T': (HID, HID), 'mlp1_b': (1, HID), 'outWT': (HID, 8), 'out_b': (1, 8),
    'featT_full': (EMB, B, N), 'featT_l': (EMB, BL, N),
    'adjbias': (BL, N, N), 'sT': (N, N, BL), 'sdiagT': (BL, N),
    'town': (1, BL),
}
for _l in range(L):
    SHAPES[f'Wc1_{_l}'] = (HID, H4)
    SHAPES[f'Wc2_{_l}'] = (HID, H2)
    SHAPES[f'bias2_{_l}'] = (1, H2)
    SHAPES[f'Wr_{_l}'] = (HID, H2 + 1)
    SHAPES[f'Wpre_{_l}'] = (HID, PRE)
    SHAPES[f'biaspre_{_l}'] = (1, PRE)


# ================================================================ device build

def _loadw(nc, pool, dram, kdim, fdim, tag, dtype=BF16):
    kc = kdim // 128
    t = pool.tile([128, kc, fdim], dtype, tag=tag)
    nd = len(dram.shape)
    if nd == 2:
        src = dram[:].rearrange("(c p) f -> p c f", p=128)
    elif nd == 3:
        src = dram[:].rearrange("(c p) a b -> p c (a b)", p=128)
    else:
        raise ValueError(nd)
    nc.gpsimd.dma_start(t[:], src)
    return t


WAIT_CAP = {}


def _cap_waits(nc):
    """Split excess semaphore waits onto same-engine NOPs (HW wait-slot caps)."""
    for f in nc.m.functions:
        for bb in f.blocks:
            newlist = []
            for ins in bb.instructions:
                si = getattr(ins, 'sync_info', None)
                waits = list(si.on_wait) if si and si.on_wait else []
                cap = WAIT_CAP.get(type(ins).__name__, 1)
                if len(waits) > cap:
                    excess = waits[:-cap] if cap > 0 else waits
                    keep = waits[-cap:] if cap > 0 else []
                    for w in excess:
                        nop = mybir.InstNoOp(
                            name=nc.get_next_instruction_name(),
                            text_hint='wait_spill', bass_nofuse=True)
                        nop.engine = ins.engine
                        nop.sync_info = mybir.SyncInfo(on_wait=[w], on_update=[])
                        nc.register_instruction(nop, overwrite=True)
                        newlist.append(nop)
                    si.on_wait = keep
                    ins.sync_info = si
                newlist.append(ins)
            bb.instructions = newlist


def build_nc():
    nc = bass.Bass()
    din = {}
    for name, shp in SHAPES.items():
        din[name] = nc.dram_tensor(name, list(shp),
                                   I32 if name == 'town' else F32,
                                   kind="ExternalInput")
    out_dram = nc.dram_tensor('out', [BL, N, NCLS], F32, kind="ExternalOutput")
    dbg_dram = (nc.dram_tensor('dbg', [L, N, BL, HID], F32, kind="ExternalOutput")
                if DEBUG else None)

    pre_dram = [nc.dram_tensor(f'pre_dram{l}', [N, 24, HID], BF16) for l in range(L)]
    hq_dram = [nc.dram_tensor(f'hq_dram{l}', [N, BL, HID], BF16) for l in range(L)]
    lstmT_dram = nc.dram_tensor('lstmT_dram', [B, 128, 4, N], BF16)

    with tile.TileContext(nc) as tc:  # noqa: SIM117
        with tc.tile_pool(name="w", bufs=1) as wpool, \
             tc.tile_pool(name="state", bufs=1) as state, \
             tc.tile_pool(name="step", bufs=3) as step, \
             tc.tile_pool(name="dma2", bufs=3) as dma2:

            ident = state.tile([128, 128], BF16, tag='ident')
            make_identity(nc, ident[:])
            ones_row = state.tile([1, 128], BF16, tag='ones')
            nc.vector.memset(ones_row[:], 1.0)

            # featT time-shares the 32KB 'lstmWih' slot: H0 reads it, then
            # the LSTM stream's WihT load takes the slot, then the MLP
            # reloads features into it again.
            featT = _loadw(nc, wpool, din['featT_l'], EMB, BL * N, 'lstmWih')
            featT4 = featT[:].rearrange("p c (b n) -> p c b n", b=BL)
            fc1T = _loadw(nc, wpool, din['fc1T'], EMB, HID, 'w8a')
            fc1b = wpool.tile([1, HID], BF16, tag='brow')
            nc.gpsimd.dma_start(fc1b[:], din['fc1_b'][:])
            sTt = wpool.tile([128, N, BL], BF16, tag='sT')
            nc.gpsimd.dma_start(sTt[:], din['sT'][:])

            HT = [state.tile([128, 4, BL, N], BF16, tag=f'HT{k}', name=f'HT{k}')
                  for k in range(3)]
            lstmTl = state.tile([128, 4, BL, N], BF16, tag='lstmTl')
            for k in range(3):
                nc.vector.memset(HT[k][:], 0.0)
            nc.vector.memset(lstmTl[:], 0.0)

            # ---- phase 1: H0; LSTM becomes a stream pumped inside the
            # recurrence loops (fills PE idle time, keeps HAM warm) ----
            psbox = {}
            lstm_gen = None
            if not KSKIP_LSTM:
                lstm_gen = _lstm_stream(nc, din, wpool, state, step, dma2,
                                        psbox, ones_row, ident, lstmT_dram)
            with tc.tile_pool(name="ps1", bufs=1, space="PSUM") as ps1:
                if not KSKIP_H0:
                    _h0_phase(nc, tc, ps1, step, featT4, fc1T, fc1b, ones_row,
                              ident, HT[0], hq_dram[0])

            # ---- phase 2: GNN layers ----
            for l in range(L if not KSKIP_PRE else 0):
                with tc.tile_pool(name=f"psp{l}", bufs=2, space="PSUM") as psp:
                    Qpre = _pre_phase(nc, tc, din, l, HT[l], wpool, step, psp,
                                      ones_row, ident, pre_dram[l], state)
                if KSKIP_REC:
                    continue
                with tc.tile_pool(name=f"psr{l}", bufs=1, space="PSUM") as psr:
                    psbox['ps'] = psr
                    _recurrence(nc, tc, din, l, Qpre, wpool, state, step, dma2,
                                psr, ident, ones_row, sTt, pre_dram[l], hq_dram,
                                HT, dbg_dram, lstm_gen)
                    # finish any half-emitted LSTM block before the PSUM
                    # pool closes (its g tile lives in this pool)
                    while lstm_gen is not None and psbox.get('mid'):
                        try:
                            next(lstm_gen)
                        except StopIteration:
                            break

            # drain any leftover LSTM chunks
            if lstm_gen is not None:
                with tc.tile_pool(name="psl", bufs=1, space="PSUM") as psl:
                    psbox['ps'] = psl
                    for _ in lstm_gen:
                        pass

            # own-rows gather: lstmT_dram[town[b]] -> lstmTl[:, :, b, :]
            town_sb = state.tile([1, BL], I32, tag='town')
            nc.gpsimd.dma_start(town_sb[:], din['town'][:])
            if KSKIP_LSTM:
                tvals = None
            else:
                _, tvals = nc.values_load_multi_w_load_instructions(
                town_sb[0:1, :], engines=[mybir.EngineType.Pool],
                min_val=0, max_val=B - 1, skip_runtime_bounds_check=True)
            if tvals is not None:
                for b in range(BL):
                    src = lstmT_dram[bass.ds(tvals[b], 1), :, :, :]
                    nc.gpsimd.dma_start(lstmTl[:, :, b, :], src)

            # ---- phase 3: final MLP ----
            with tc.tile_pool(name="psm", bufs=2, space="PSUM") as psm:
                if KSKIP_MLP:
                    for b in range(BL):
                        nc.gpsimd.dma_start(out_dram[b, :, :],
                                            ident[0:128, 0:NCLS])
                else:
                    _final_mlp(nc, tc, din, wpool, step, psm, HT, lstmTl,
                           ones_row, ident, out_dram)
    _cap_waits(nc)
    return nc


def _h0_phase(nc, tc, ps, step, featT4, fc1T, fc1b, ones_row, ident, HT0,
              hq0_dram):
    for b in range(BL):
        p = ps.tile([128, HID], F32, tag='h0ps')
        for k in range(8):
            nc.tensor.matmul(p[:], featT4[:, k, b, :], fc1T[:, k, :],
                             start=(k == 0), stop=False)
        nc.tensor.matmul(p[:], ones_row[0:1, 0:128], fc1b[:],
                         start=False, stop=True)
        h0 = step.tile([128, HID], BF16, tag='As', bufs=2)
        nc.scalar.activation(h0[:], p[:], AF.Relu)
        for c in range(4):
            tp = ps.tile([128, 128], BF16, tag='h0tp')
            nc.tensor.transpose(tp[:], h0[:, c * 128:(c + 1) * 128], ident[:])
            nc.vector.tensor_copy(HT0[:, c, b, :], tp[:])
        nc.gpsimd.dma_start(hq0_dram[:, b, :], h0[:])


def _pump(gen, n=1):
    if gen is None:
        return
    for _ in range(n):
        try:
            next(gen)
        except StopIteration:
            return


def _lstm_stream(nc, din, wpool, state, step, dma2, psbox, ones_row, ident,
                 lstmT_dram):
    """Generator emitting the LSTM in self-contained chunks so it can be
    interleaved into the GNN recurrence (fills PE idle time + keeps the
    HAM clock-gate warm). Uses tanh-only activations (exp table set)."""
    WihT = _loadw(nc, wpool, din['lstm_WihT'], EMB, 4 * HID, 'lstmWih')
    WhhT = _loadw(nc, wpool, din['lstm_WhhT'], HID, 4 * HID, 'lstmWhh')
    lb = wpool.tile([1, 4 * HID], BF16, tag='brow_lb')
    nc.gpsimd.dma_start(lb[:], din['lstm_b'][:])

    hT = state.tile([128, 4, 128], BF16, tag='lhT')     # h_{t-1} transposed
    cst = state.tile([128, HID], F32, tag='lc')
    nc.vector.memset(hT[:], 0.0)
    nc.vector.memset(cst[:], 0.0)
    yield

    for t in range(B):
        ft = dma2.tile([128, 8, 128], BF16, tag='lft', bufs=2)
        nc.gpsimd.dma_start(
            ft[:], din['featT_full'][:, t, :].rearrange("(c p) n -> p c n", p=128))
        yield
        # gate order [i f o g~]; tanh-domain for the three sigma gates
        gt = []
        for blk in range(4):
            sl = slice(blk * HID, (blk + 1) * HID)
            g = psbox['ps'].tile([128, HID], F32, tag='lg', bufs=2)
            for k in range(8):
                nc.tensor.matmul(g[:], ft[:, k, :], WihT[:, k, sl],
                                 start=(k == 0), stop=False)
                if k == 3:
                    psbox['mid'] = True
                    yield
            for k in range(4):
                nc.tensor.matmul(g[:], hT[:, k, :], WhhT[:, k, sl],
                                 start=False, stop=False)
            nc.tensor.matmul(g[:], ones_row[0:1, 0:128], lb[:, sl],
                             start=False, stop=True)
            gs = step.tile([128, HID], BF16, tag=f'lgt{blk}', bufs=2)
            nc.scalar.activation(gs[:], g[:], AF.Tanh,
                                 scale=(0.5 if blk < 3 else 1.0))
            gt.append(gs)
            psbox['mid'] = False
            yield
        sig = []
        for blk in range(3):
            sg = step.tile([128, HID], BF16, tag=f'lgt{blk}', bufs=2)
            nc.vector.tensor_scalar(sg[:], gt[blk][:], 0.5, 0.5,
                                    op0=ALU.mult, op1=ALU.add)
            sig.append(sg)
        yield
        m1 = step.tile([128, HID], F32, tag='lm1', bufs=2)
        nc.vector.tensor_tensor(m1[:], sig[1][:], cst[:], op=ALU.mult)
        m2 = step.tile([128, HID], BF16, tag='lm2', bufs=2)
        nc.vector.tensor_tensor(m2[:], sig[0][:], gt[3][:], op=ALU.mult)
        nc.vector.tensor_tensor(cst[:], m1[:], m2[:], op=ALU.add)
        tct = step.tile([128, HID], BF16, tag='lgt3', bufs=2)
        nc.scalar.activation(tct[:], cst[:], AF.Tanh)
        hsb = step.tile([128, HID], BF16, tag='lm2', bufs=2)
        nc.vector.tensor_tensor(hsb[:], sig[2][:], tct[:], op=ALU.mult)
        yield
        for c in range(4):
            tp = psbox['ps'].tile([128, 128], BF16, tag='lg', bufs=2)
            nc.tensor.transpose(tp[:], hsb[:, c * 128:(c + 1) * 128], ident[:])
            nc.vector.tensor_copy(hT[:, c, :], tp[:])
        nc.gpsimd.dma_start(lstmT_dram[t, :, :, :], hT[:])
        yield


def _pre_phase(nc, tc, din, l, HTl, wpool, step, ps, ones_row, ident,
               pre_dram_l, state):
    Wpre = _loadw(nc, wpool, din[f'Wpre_{l}'], HID, PRE, 'bigw')
    bpre = wpool.tile([1, PRE], BF16, tag='brow')
    nc.gpsimd.dma_start(bpre[:], din[f'biaspre_{l}'][:])

    pq = ps.tile([128, BL], F32, tag='pq', bufs=1)
    for b in range(BL):
        for blk in range(6):
            sl = slice(blk * HID, (blk + 1) * HID)
            p = ps.tile([128, HID], F32, tag='pp')
            for k in range(4):
                nc.tensor.matmul(p[:], HTl[:, k, b, :], Wpre[:, k, sl],
                                 start=(k == 0), stop=False)
            nc.tensor.matmul(p[:], ones_row[0:1, 0:128], bpre[:, sl],
                             start=False, stop=True)
            sb = step.tile([128, HID], BF16, tag='psb', bufs=2)
            nc.vector.tensor_copy(sb[:], p[:])
            j = 4 * blk + b if blk < 4 else (16 + b if blk == 4 else 20 + b)
            nc.gpsimd.dma_start(pre_dram_l[:, j, :], sb[:])
        for k in range(4):
            nc.tensor.matmul(pq[:, b:b + 1], HTl[:, k, b, :],
                             Wpre[:, k, PRE - 1:PRE],
                             start=(k == 0), stop=False)
        nc.tensor.matmul(pq[:, b:b + 1], ones_row[0:1, 0:128],
                         bpre[:, PRE - 1:PRE], start=False, stop=True)
    # Qpre transpose -> [BL, 128] fp32 in state
    qsb = step.tile([128, BL], BF16, tag='pqsb')
    nc.vector.tensor_copy(qsb[:], pq[:])
    qt = ps.tile([BL, 128], BF16, tag='pqT', bufs=1)
    nc.tensor.transpose(qt[:], qsb[:], ident[:])
    Qpre = state.tile([BL, N], F32, tag='Qpre')
    nc.vector.tensor_copy(Qpre[:], qt[:])
    return Qpre


def _warm(nc, ident, scratch, rhs_ap):
    """HAM pacemaker: a tiny matmul whose rhs is a just-produced chain
    tile. Fires right after that op completes, keeping the PE activity
    window busy through long DVE/ACT stretches (else K drops to 4/8 and
    every real matmul runs at half clock)."""
    nc.tensor.matmul(scratch[0:4, 256:512], ident[0:BL, 0:BL],
                     rhs_ap, start=True, stop=True)


def _recurrence(nc, tc, din, l, Qpre, wpool, state, step, dma2, ps, ident,
                ones_row, sTt, pre_dram_l, hq_dram, HT, dbg_dram,
                lstm_gen=None):
    Wc1 = _loadw(nc, wpool, din[f'Wc1_{l}'], HID, H4, 'w16')
    Wc2 = _loadw(nc, wpool, din[f'Wc2_{l}'], HID, H2, 'w8a')
    Wrt = _loadw(nc, wpool, din[f'Wr_{l}'], HID, H2 + 1, 'Wr')
    b2 = wpool.tile([1, H2], BF16, tag='b2')
    nc.gpsimd.dma_start(b2[:], din[f'bias2_{l}'][:])
    V01 = [state.tile([128, H2], BF16, tag=f'V_{b}', name=f'V{l}_{b}')
           for b in range(BL)]
    Kneg = state.tile([BL, N], F32, tag='Kneg')
    ew = state.tile([BL, N], BF16, tag='ew')
    Wz0 = state.tile([128, 16], BF16, tag='Wz0')
    Wz1 = state.tile([128, 16], BF16, tag='Wz1')
    aneg = state.tile([BL, N], F32, tag='aneg')
    for t in (ew, Wz0, Wz1):
        nc.vector.memset(t[:], 0.0)
    for t in V01:
        nc.vector.memset(t[:], 0.0)
    nc.vector.memset(Kneg[:], 0.0)

    HTnext = HT[l + 1]
    sdT = state.tile([BL, N], F32, tag='sdT')
    nc.sync.dma_start(sdT[:], din['sdiagT'][:])
    Vsb_prev = None

    for i in range(KNSTEP):
        # ---------- prefetches ----------
        qt = dma2.tile([128, HID], BF16, tag='qt', bufs=2)
        if KSIMINIT:
            nc.gpsimd.memset(qt[:], 0.0)
        nc.sync.dma_start(qt[64:68, :], hq_dram[l][i, :, :])
        abt = dma2.tile([BL, N], F32, tag='abt', bufs=2)
        nc.sync.dma_start(abt[:], din['adjbias'][:, i, :])
        pA = dma2.tile([128, HID], BF16, tag='pA', bufs=2)
        if KSIMINIT:
            nc.gpsimd.memset(pA[:], 0.0)
        for s in range(4):
            nc.sync.dma_start(pA[32 * s:32 * s + 4, :],
                              pre_dram_l[i, 4 * s:4 * s + 4, :])
        Mps = ps.tile([128, HID], F32, tag='Mps')
        VpsK = ps.tile([128, 512], F32, tag='VpsK')
        shr = ps.tile([128, 40], BF16, tag='shr')
        if KSIMINIT:
            nc.vector.memset(shr[:], 0.0)

        Msb = qt[0:4, :]   # M and q share one tile: rows 0:4 / 64:68
        if i > 0:
            # ---------- attention ----------
            nc.vector.scalar_tensor_tensor(
                aneg[:, 0:i], Kneg[:, 0:i], Qpre[:, i:i + 1], abt[:, 0:i],
                op0=ALU.subtract, op1=ALU.subtract)
            mneg = step.tile([BL, 1], F32, tag='mneg')
            nc.vector.tensor_reduce(mneg[:], aneg[:, 0:i], axis=AX.X,
                                    op=ALU.min)
            # unnormalized weights; 1/Z is folded into the Msb cast below
            zs = step.tile([BL, 1], F32, tag='zs')
            nc.scalar.activation(ew[:, 0:i], aneg[:, 0:i], AF.Exp,
                                 bias=mneg[:], scale=-1.0, accum_out=zs[:])
            rz = step.tile([BL, 1], F32, tag='rz')
            nc.vector.reciprocal(rz[:], zs[:])
            # lag-1 correction weights: w0c = w_{i-1}*s, w1c = w_{i-1}-w0c
            w0c = step.tile([BL, 1], F32, tag='w0c')
            w1c = step.tile([BL, 1], F32, tag='w1c')
            nc.gpsimd.tensor_tensor(w0c[:], ew[:, i - 1:i], sdT[:, i:i + 1],
                                    op=ALU.mult)
            nc.gpsimd.tensor_tensor(w1c[:], ew[:, i - 1:i], w0c[:],
                                    op=ALU.subtract)
            D0 = step.tile([BL, BL], BF16, tag='D0', bufs=2)
            D1 = step.tile([BL, BL], BF16, tag='D1', bufs=2)
            nc.gpsimd.tensor_scalar_mul(D0[:], ident[0:BL, 0:BL], w0c[:])
            nc.gpsimd.tensor_scalar_mul(D1[:], ident[0:BL, 0:BL], w1c[:])
            # transpose ew -> eT [128, 4] (shared psum cols 0:4)
            nc.tensor.transpose(shr[:, 0:4], ew[:], ident[0:BL, 0:BL])
            # w0/w1 diagonal scatter
            nc.vector.tensor_tensor(_diag(Wz0), shr[:, 0:4], sTt[:, i, :],
                                    op=ALU.mult)
            nc.vector.tensor_tensor(_diag(Wz1), shr[:, 0:4], _diag(Wz0),
                                    op=ALU.subtract)
            # ---------- M (single accumulation group) --------------------
            # main matmuls only read V rows j<=i-2 (K-sliced): the row
            # written by last step's scatter DMA is excluded, so the DMA
            # has a full step of slack. j=i-1 enters via D0/D1 * Vsb_prev.
            ke = i - 1
            nc.tensor.matmul(Mps[0:4, :], D0[:], Vsb_prev[:, 0:HID],
                             start=True, stop=False)
            nc.tensor.matmul(Mps[0:4, :], D1[:], Vsb_prev[:, HID:H2],
                             start=False, stop=(ke == 0))
            for b in range(BL if ke > 0 else 0):
                nc.tensor.matmul(Mps[0:4, :], Wz0[0:ke, 4 * b:4 * b + 4],
                                 V01[b][0:ke, 0:HID], start=False,
                                 stop=False)
                nc.tensor.matmul(Mps[0:4, :], Wz1[0:ke, 4 * b:4 * b + 4],
                                 V01[b][0:ke, HID:H2], start=False,
                                 stop=(b == BL - 1))
            # Msb lands in rows 0:4 of the qt tile so the combine ops can
            # row-stack the C-GRU (M) and P-GRU (q) operands in one op;
            # the softmax 1/Z lands here as a per-partition scalar
            nc.vector.tensor_scalar_mul(Msb[:], Mps[0:4, :], rz[:])
            for c in range(4):
                nc.tensor.transpose(shr[:, 4 + 4 * c:8 + 4 * c],
                                    Msb[:, 128 * c:128 * (c + 1)],
                                    ident[0:BL, 0:BL])
            MT = step.tile([128, 16], BF16, tag='MT')
            nc.vector.tensor_copy(MT[:], shr[:, 4:20])
        else:
            nc.vector.memset(Msb[:], 0.0)

        # ---------- gates ----------
        # Y rows 0:4 = psB_C (r_C-multiplied), rows 64:68 = pre pWhh_n
        # Z rows 0:4 = pre cWih_n,             rows 64:68 = psB_P
        # so ntin = Asig*Y + Z runs as two row-stacked [0:68] DVE ops.
        Yt = step.tile([128, HID], BF16, tag='Yt', bufs=2)
        Zt = step.tile([128, HID], BF16, tag='Zt', bufs=2)
        nc.sync.dma_start(Yt[64:68, :], pre_dram_l[i, 20:24, :])
        nc.sync.dma_start(Zt[0:4, :], pre_dram_l[i, 16:20, :])
        psA = ps.tile([128, HID], F32, tag='psA')
        psB = ps.tile([128, HID], F32, tag='psB')
        if KSIMINIT:
            nc.vector.memset(psA[:], 0.0)
            nc.vector.memset(psB[:], 0.0)
        # bias first (no MT dependency)
        for si, s in enumerate((0, 64)):
            nc.tensor.matmul(psB[s:s + 4, :], ones_row[0:1, 0:4],
                             b2[:, HID * si:HID * (si + 1)],
                             start=True, stop=(i == 0), tile_position=(0, s))
        if i > 0:
            for k in range(4):
                for si, s in enumerate((0, 64)):
                    nc.tensor.matmul(psB[s:s + 4, :], MT[:, 4 * k:4 * k + 4],
                                     Wc2[:, k, HID * si:HID * (si + 1)],
                                     start=False, stop=(k == 3),
                                     tile_position=(0, s))
            nc.scalar.activation(Yt[0:4, :], psB[0:4, :], AF.Copy)
            nc.scalar.activation(Zt[64:68, :], psB[64:68, :], AF.Copy)
            for k in range(4):
                for s in range(4):
                    nc.tensor.matmul(psA[32 * s:32 * s + 4, :],
                                     MT[:, 4 * k:4 * k + 4],
                                     Wc1[:, k, HID * s:HID * (s + 1)],
                                     start=(k == 0), stop=(k == 3),
                                     tile_position=(0, 32 * s))

        # sigma(x) = 0.5 + 0.5*tanh(x/2): keeps ACT on the exp table set
        # (sigmoid lives in a different set -> 2.7us table swap per use)
        _pump(lstm_gen)
        Ath = step.tile([128, HID], BF16, tag='Ath', bufs=2)
        if i > 0:
            Atn = step.tile([128, HID], BF16, tag='Atn', bufs=2)
            nc.vector.tensor_tensor(Atn[:], psA[:], pA[:], op=ALU.add)
            nc.scalar.activation(Ath[:], Atn[:], AF.Tanh, scale=0.5)
        else:
            nc.scalar.activation(Ath[:], pA[:], AF.Tanh, scale=0.5)
        # Asig rows {0:4,64:68} = r gates; Az rows {0:4,64:68} = z gates
        # (shifted from Ath rows 32:100) so combine ops stay row-aligned
        Asig = step.tile([128, HID], BF16, tag='Asig', bufs=2)
        nc.vector.tensor_scalar(Asig[0:68, :], Ath[0:68, :], 0.5, 0.5,
                                op0=ALU.mult, op1=ALU.add)
        Az = step.tile([128, HID], BF16, tag='Az', bufs=2)
        nc.vector.tensor_scalar(Az[0:4, :], Ath[32:36, :], 0.5, 0.5,
                                op0=ALU.mult, op1=ALU.add)
        nc.vector.tensor_scalar(Az[64:68, :], Ath[96:100, :], 0.5, 0.5,
                                op0=ALU.mult, op1=ALU.add)
        _warm(nc, ident, VpsK, Asig[0:4, 0:256])
        if i == 0:
            nc.scalar.activation(Yt[0:4, :], psB[0:4, :], AF.Copy)
            nc.scalar.activation(Zt[64:68, :], psB[64:68, :], AF.Copy)
        ntin = step.tile([128, HID], BF16, tag='ntin', bufs=2)
        if KSIMINIT:
            nc.vector.memset(ntin[:], 0.0)
        nc.vector.tensor_tensor(ntin[0:68, :], Asig[0:68, :], Yt[0:68, :],
                                op=ALU.mult)
        nc.vector.tensor_tensor(ntin[0:68, :], ntin[0:68, :], Zt[0:68, :],
                                op=ALU.add)
        _warm(nc, ident, VpsK, ntin[0:4, 0:256])
        Nt = step.tile([128, HID], BF16, tag='Nt', bufs=2)
        nc.scalar.activation(Nt[0:68, :], ntin[0:68, :], AF.Tanh)
        _pump(lstm_gen)
        _warm(nc, ident, VpsK, Nt[0:4, 0:256])
        # ---------- combine (row-stacked C|P chains) ----------
        # qt rows 0:4 = M, rows 64:68 = q  ->  h = n + z*(x - n) per GRU
        cmb = step.tile([128, HID], BF16, tag='cmb', bufs=2)
        nc.vector.tensor_tensor(cmb[0:68, :], qt[0:68, :], Nt[0:68, :],
                                op=ALU.subtract)
        nc.vector.tensor_tensor(cmb[0:68, :], Az[0:68, :], cmb[0:68, :],
                                op=ALU.mult)
        _warm(nc, ident, VpsK, cmb[0:4, 0:256])
        hC = step.tile([128, HID], BF16, tag='hC', bufs=2)
        nc.vector.tensor_tensor(hC[0:68, :], Nt[0:68, :], cmb[0:68, :],
                                op=ALU.add)
        _warm(nc, ident, VpsK, hC[0:4, 0:256])
        hbf = step.tile([128, HID], BF16, tag='hbf', bufs=2)
        nc.vector.tensor_copy(hbf[0:4, :], hC[64:68, :])
        nc.vector.tensor_tensor(hbf[0:4, :], hC[0:4, :], hbf[0:4, :],
                                op=ALU.add)
        # ---------- h transposes + Wr ----------
        for c in range(4):
            nc.tensor.transpose(shr[:, 20 + 4 * c:24 + 4 * c],
                                hbf[0:4, 128 * c:128 * (c + 1)],
                                ident[0:BL, 0:BL])
        nc.vector.tensor_copy(HTnext[:, :, :, i], shr[:, 20:36])
        # V0 / V1 / K in three concurrent PE col-tile groups; K gets its
        # own PSUM tile so the Kneg copy (-> next step's softmax) does not
        # wait on the V accumulations (bank-level deps).
        Vps = ps.tile([128, HID], F32, tag='Vps')
        for k in range(4):
            nc.tensor.matmul(VpsK[64:68, 0:1], HTnext[:, k, :, i],
                             Wrt[:, k, H2:H2 + 1],
                             start=(k == 0), stop=(k == 3),
                             tile_position=(0, 64))
        nc.vector.tensor_copy(Kneg[:, i:i + 1], VpsK[64:68, 0:1])
        for k in range(4):
            nc.tensor.matmul(Vps[0:4, :], HTnext[:, k, :, i], Wrt[:, k, 0:HID],
                             start=(k == 0), stop=(k == 3),
                             tile_position=(0, 0))
        for k in range(4):
            nc.tensor.matmul(Vps[32:36, :], HTnext[:, k, :, i],
                             Wrt[:, k, HID:H2], start=(k == 0), stop=(k == 3),
                             tile_position=(0, 32))
        Vsb = step.tile([BL, H2], BF16, tag='Vsb', bufs=2)
        nc.scalar.activation(Vsb[:, 0:HID], Vps[0:4, :], AF.Copy)
        nc.scalar.activation(Vsb[:, HID:H2], Vps[32:36, :], AF.Copy)
        Vsb_prev = Vsb
        _pump(lstm_gen)
        for b in range(BL):
            nc.sync.dma_start(V01[b][i:i + 1, :], Vsb[b:b + 1, :])
        if l + 1 < L:
            nc.sync.dma_start(hq_dram[l + 1][i, :, :], hbf[0:4, :])
        if dbg_dram is not None:
            hdb = step.tile([BL, HID], F32, tag='hdb')
            nc.vector.tensor_copy(hdb[:], hbf[0:4, :])
            nc.gpsimd.dma_start(dbg_dram[l, i, :, :], hdb[:])


def _diag(Wz):
    """Diagonal columns {0,5,10,15} of a [128,16] tile as a [128,4] AP."""
    ap = Wz[:]
    return bass.AP(tensor=ap.tensor, offset=ap.offset,
                   ap=[ap.ap[0], [5, 4]])


def _final_mlp(nc, tc, din, wpool, step, ps, HT, lstmTl, ones_row,
               ident, out_dram):
    featT = _loadw(nc, wpool, din['featT_l'], EMB, BL * N, 'lstmWih')
    featT4 = featT[:].rearrange("p c (b n) -> p c b n", b=BL)
    mlp0T = _loadw(nc, wpool, din['mlp0T'], 4 * HID + EMB, HID, 'bigw')
    mlp1T = _loadw(nc, wpool, din['mlp1T'], HID, HID, 'mlp1T')
    outWT = _loadw(nc, wpool, din['outWT'], HID, 8, 'outWT')
    b0 = wpool.tile([1, HID], BF16, tag='brow')
    nc.gpsimd.dma_start(b0[:], din['mlp0_b'][:])
    b1 = wpool.tile([1, HID], BF16, tag='brow_b1')
    nc.gpsimd.dma_start(b1[:], din['mlp1_b'][:])
    bo = wpool.tile([1, 8], BF16, tag='brow2')
    nc.gpsimd.dma_start(bo[:], din['out_b'][:])

    for b in range(BL):
        p1 = ps.tile([128, HID], F32, tag='fp1')
        kc = 0
        for blk in range(3):
            for c in range(4):
                nc.tensor.matmul(p1[:], HT[blk][:, c, b, :], mlp0T[:, kc, :],
                                 start=(kc == 0), stop=False)
                kc += 1
        for k in range(8):
            nc.tensor.matmul(p1[:], featT4[:, k, b, :], mlp0T[:, kc, :],
                             start=False, stop=False)
            kc += 1
        for c in range(4):
            nc.tensor.matmul(p1[:], lstmTl[:, c, b, :], mlp0T[:, kc, :],
                             start=False, stop=False)
            kc += 1
        nc.tensor.matmul(p1[:], ones_row[0:1, 0:128], b0[:],
                         start=False, stop=True)
        x1 = step.tile([128, HID], BF16, tag='As', bufs=2)
        nc.scalar.activation(x1[:], p1[:], AF.Relu)
        x1T = step.tile([128, 4, 128], BF16, tag='Asig', bufs=2)
        for c in range(4):
            tp = ps.tile([128, 128], BF16, tag='ftp')
            nc.tensor.transpose(tp[:], x1[:, 128 * c:128 * (c + 1)], ident[:])
            nc.vector.tensor_copy(x1T[:, c, :], tp[:])
        p2 = ps.tile([128, HID], F32, tag='fp2')
        for k in range(4):
            nc.tensor.matmul(p2[:], x1T[:, k, :], mlp1T[:, k, :],
                             start=(k == 0), stop=False)
        nc.tensor.matmul(p2[:], ones_row[0:1, 0:128], b1[:],
                         start=False, stop=True)
        x2 = step.tile([128, HID], BF16, tag='hbf', bufs=2)
        nc.scalar.activation(x2[:], p2[:], AF.Relu)
        x2T = step.tile([128, 4, 128], BF16, tag='Nt', bufs=2)
        for c in range(4):
            tp = ps.tile([128, 128], BF16, tag='ftp')
            nc.tensor.transpose(tp[:], x2[:, 128 * c:128 * (c + 1)], ident[:])
            nc.vector.tensor_copy(x2T[:, c, :], tp[:])
        po = ps.tile([128, 8], F32, tag='fpo')
        for k in range(4):
            nc.tensor.matmul(po[:], x2T[:, k, :], outWT[:, k, :],
                             start=(k == 0), stop=False)
        nc.tensor.matmul(po[:], ones_row[0:1, 0:128], bo[:],
                         start=False, stop=True)
        ot = step.tile([128, NCLS], F32, tag='Msb', bufs=2)
        nc.vector.tensor_copy(ot[:], po[:, 0:NCLS])
        nc.gpsimd.dma_start(out_dram[b, :, :], ot[:])


# ================================================================ entry point

_NC_CACHE = {}


def kernel(**inputs):
    maps = prep_inputs(inputs)
    if 'nc' not in _NC_CACHE:
        _NC_CACHE['nc'] = build_nc()
    nc = _NC_CACHE['nc']
    res = run_bass_kernel_spmd(nc, maps, list(range(NCORES)))
    out = np.concatenate([res.results[c]['out'] for c in range(NCORES)], axis=0)
    return out.astype(np.float32)


def kernel_debug(**inputs):
    assert DEBUG
    maps = prep_inputs(inputs)
    nc = build_nc()
    res = run_bass_kernel_spmd(nc, maps, list(range(NCORES)))
    out = np.concatenate([res.results[c]['out'] for c in range(NCORES)], axis=0)
    dbg = np.stack([res.results[c]['dbg'] for c in range(NCORES)], axis=0)
    return out.astype(np.float32), dbg

